# revision 28
# baseline (speedup 1.0000x reference)
"""Trainium2 Bass kernel for nn_CropAndPadMask (paste instance masks into canvases).

Math: for each (b, n) box the reference output is a bilinear resize of the
28x28 mask pasted into a zero [H, W] canvas.  Every non-zero output value
lies inside the box window [ymin, ymax) x [xmin, xmax) with extent
ph = ymax-ymin <= 200, pw = xmax-xmin <= 200.  Instead of streaming 256 MiB
of mostly-zero canvases, each core computes one f16 patch per box:

    patch[b, n] = WyP[b, n] @ mask[b, n] @ WxP[b, n]

and the host pastes patches into a zero canvas while gathering (pure data
movement).  Device HBM traffic drops ~30x vs the full canvases.

Box sizes are data dependent but the SPMD program must have static shapes,
so kernel() SPECIALIZES the program to the input: the 256 boxes are sorted
by (ph, pw) descending and dealt round-robin - sorted rank 8c+k goes to
core k, slot c.  All 8 cores share slot shapes (max over the 8 near-equal
boxes) and get equal work.  Groups of 4 slots share one [ph_g, pw_g] shape
(max over the group) so each group is written by a single DMA with uniform
strides.

Per box the PE work is limited by LDWEIGHTS (~57 + 1.18*M ns per matmul),
so the schedule minimizes matmul count and total output rows:
- S = mask @ WxP for the 2 boxes of a slot-pair lands at PE tile rows
  q0/q32 of ONE [64, pw_g] PSUM tile (they execute concurrently on the PE
  quads); one PSUM->SBUF copy serves both.
- ph_g <= 128: the patch is ONE matmul [ph_g, pw_g] (partition = canvas row).
- ph_g  > 128: two matmuls (even/odd canvas rows) into [ph_g/2, 2*pw_g],
  which also doubles the DMA descriptor width.

f16 off-PSUM everywhere: PE at 1 cycle/col, DMA bytes halved, and f16
rounding (~4e-4 rel) is far inside the 2e-2 gate.
"""

import sys

for _p in ("/opt/trn_rl_repo", "/root/.axon_site/_ro/trn_rl_repo"):
    if _p not in sys.path:
        sys.path.append(_p)

import numpy as np

import concourse.bass as bass
import concourse.mybir as mybir
import concourse.tile as tile
from concourse.bass_utils import run_bass_kernel_spmd

B, N, H, W, MH, MW = 4, 64, 512, 512, 28, 28
N_CORES = 8
CPC = (B * N) // N_CORES  # boxes per core = 32
GQ = 8                    # slots per group (one out-DMA per group; wide
                          # groups => multi-KB DMA descriptors)
NG = CPC // GQ            # groups per core = 4
FREE = 496                # fixed input row: WxP @0 | bd-maskT @200 | wyT @296
TRI = (CPC + 2) // 3      # S-triples per core (last may be ragged)


def _split_multi_waits(nc: bass.Bass) -> None:
    """The walrus bundled in this container accepts at most ONE sync wait per
    instruction.  Tile freely attaches several.  Hoist the extras onto
    standalone EventSemaphore carriers inserted just before the instruction on
    the same engine (per-engine program order makes this equivalent)."""
    n_new = 0
    for f in nc.m.functions:
        for bb in f.blocks:
            lst = bb.instructions
            i = 0
            while i < len(lst):
                ins = lst[i]
                si = ins.sync_info
                if si is not None and si.on_wait and len(si.on_wait) > 1:
                    waits = list(si.on_wait)
                    ins.sync_info = mybir.SyncInfo(
                        on_wait=waits[:1], on_update=list(si.on_update or [])
                    )
                    carriers = []
                    for w in waits[1:]:
                        n_new += 1
                        carriers.append(
                            mybir.InstEventSemaphore(
                                name=f"I-waitsplit-{n_new}",
                                ins=[],
                                outs=[],
                                engine=ins.engine,
                                sync_info=mybir.SyncInfo(on_wait=[w], on_update=[]),
                            )
                        )
                    lst[i:i] = carriers
                    i += len(carriers)
                i += 1


def _layout(slot_ph, slot_pw):
    """Per-slot quantized shapes -> group meta + per-slot meta.

    Group g (GQ slots) shares (mode, rows_g, cols_g) for its stage tile and
    out-DMA; each slot keeps its own matmul/copy extents (rows_c, cols_c).
    """
    groups = []
    slots = []
    off = 0
    for g in range(CPC // GQ):
        sl = slice(GQ * g, GQ * (g + 1))
        ph_g = max(slot_ph[sl])
        eo = ph_g > 128
        rows_g = cols_g = 0
        for c in range(GQ * g, GQ * (g + 1)):
            ph_c, pw_c = slot_ph[c], slot_pw[c]
            if eo:
                rows_c, cols_c = ph_c // 2, 2 * pw_c
            else:
                rows_c, cols_c = ph_c, pw_c
            slots.append((ph_c, pw_c, rows_c, cols_c))
            rows_g = max(rows_g, rows_c)
            cols_g = max(cols_g, cols_c)
        groups.append(("eo" if eo else "single", rows_g, cols_g, off))
        off += GQ * rows_g * cols_g
    return groups, slots, off


def build_nc(plan, cpc: int = CPC) -> bass.Bass:
    """One core's program, specialized to the plan's slot/group shapes."""
    f16 = mybir.dt.float16
    f32 = mybir.dt.float32
    groups, slots, out_elems = plan.gmeta, plan.smeta, plan.out_elems
    ng = len(groups)
    nc = bass.Bass()
    wmat = nc.dram_tensor(
        "wmat", [3, 32, TRI, FREE], f16, kind="ExternalInput"
    )
    out = nc.dram_tensor("out", [out_elems], f16, kind="ExternalOutput")

    with tile.TileContext(nc) as tc:
        with (
            tc.tile_pool(name="win", bufs=3) as win_pool,
            tc.tile_pool(name="ssb", bufs=5) as s_pool,
            tc.tile_pool(name="stage", bufs=3) as stage_pool,
            tc.tile_pool(name="psum_s", bufs=4, space="PSUM") as psum_s,
            tc.tile_pool(name="psum_c", bufs=4, space="PSUM") as psum_c,
        ):
            # input loads: fixed [32, FREE] slot rows -> same 3-DMA plan
            # regardless of the patch shapes.  Box c = (pair j = c//2,
            # e = c%2) lives at SBUF partition 32*e + p of pair-tile column j.
            # wmat is stored partition-major ([2, 32, npairs, FREE]) so each
            # load descriptor covers a whole (pairs x FREE) run per partition
            # (multi-KB descriptors; the DMA engines are descriptor-rate
            # limited at ~100 ns/descriptor, so fat descriptors are crucial).
            load_plan = [
                (0, 1, "scalar"),
                (1, 3, "sync"),
                (4, TRI - 4, "gpsimd"),
            ]
            win_of: dict[int, tuple[object, int]] = {}
            for t0, tlen, eng in load_plan:
                w_t = win_pool.tile([96, tlen, FREE], f16)
                src = wmat[:, :, t0 : t0 + tlen, :].rearrange(
                    "e p t f -> (e p) t f"
                )
                getattr(nc, eng).dma_start(w_t[:], src)
                for tt in range(tlen):
                    win_of[t0 + tt] = (w_t, tt)

            s_of: dict[int, object] = {}
            eng_load = [0.0, 0.0]  # projected busy ns: [Act, DVE]

            def copy_rr(dst, src, free):
                """Greedy-balance PSUM->SBUF copies over the two PSUM-capable
                engines (GPSIMD cannot access PSUM)."""
                est = ((free + 172) * 0.833, (free + 120) * 1.042)
                if eng_load[0] + est[0] <= eng_load[1] + est[1]:
                    nc.scalar.copy(dst, src)
                    eng_load[0] += est[0]
                else:
                    nc.vector.tensor_copy(dst, src)
                    eng_load[1] += est[1]

            def s_triple(t):
                """Block-diag S = mask @ WxP for the <=3 boxes of triple t:
                ONE matmul [96, pw_t] (lhsT = block-diagonal maskT) and ONE
                PSUM->SBUF copy serve all three boxes."""
                if t >= TRI or t in s_of:
                    return
                cs = [c for c in (3 * t, 3 * t + 1, 3 * t + 2) if c < cpc]
                pw_t = max(slots[c][1] for c in cs)
                w_t, tt = win_of[t]
                s_p = psum_s.tile([96, pw_t], f32)
                nc.tensor.matmul(
                    s_p[:, :],
                    w_t[0:96, tt, 200:296],
                    w_t[0:96, tt, 0:pw_t],
                    start=True,
                    stop=True,
                )
                s_sb = s_pool.tile([96, pw_t], f16)
                copy_rr(s_sb[:], s_p[:], pw_t)
                s_of[t] = s_sb

            PIPE = 2  # triples of S lookahead
            for t in range(PIPE):
                s_triple(t)
            for k in range(ng):
                mode, rows_g, cols_g, off = groups[k]
                stage = stage_pool.tile([rows_g, GQ, cols_g], f16, tag="stage")
                for q in range(GQ):
                    c = GQ * k + q
                    j, e = c // 3, c % 3
                    s_triple(j + PIPE)
                    w_t, jj = win_of[j]
                    s_sb = s_of[j]
                    b0 = 32 * e
                    ph_c, pw_c, rows_c, cols_c = slots[c]
                    p_c = psum_c.tile([rows_c, cols_c], f32)
                    if mode == "single":
                        nc.tensor.matmul(
                            p_c[:, :],
                            w_t[b0 : b0 + MW, jj, 296 : 296 + ph_c],
                            s_sb[b0 : b0 + MW, 0:pw_c],
                            start=True,
                            stop=True,
                        )
                    else:
                        nc.tensor.matmul(
                            p_c[:, 0:pw_c],
                            w_t[b0 : b0 + MW, jj, 296 : 296 + rows_c],
                            s_sb[b0 : b0 + MW, 0:pw_c],
                            start=True,
                            stop=True,
                        )
                        nc.tensor.matmul(
                            p_c[:, pw_c : 2 * pw_c],
                            w_t[b0 : b0 + MW, jj, 396 : 396 + rows_c],
                            s_sb[b0 : b0 + MW, 0:pw_c],
                            start=True,
                            stop=True,
                        )
                    copy_rr(stage[0:rows_c, q, 0:cols_c], p_c[:], cols_c)
                # partition-row-major DRAM layout: one descriptor per
                # partition covering all GQ boxes' row p (GQ*cols_g*2 bytes)
                out_ap = out[off : off + GQ * rows_g * cols_g].rearrange(
                    "(p c j) -> p c j", c=GQ, p=rows_g
                )
                nc.sync.dma_start(out_ap, stage[:])
    _split_multi_waits(nc)
    return nc


def _box_weight_matrices(det_outs: np.ndarray):
    """Wy [BN, H, MH], Wx [BN, MW, W] (f32) + box corners, reference semantics."""
    det = np.asarray(det_outs, dtype=np.float32).reshape(B * N, 6)
    score = det[:, 5]
    thr = np.float32(50.0) if np.max(score) > 50.0 else np.float32(-100.0)
    valid = score >= thr
    box = np.maximum(det, np.float32(1.0))
    cx, cy, w, h = box[:, 0], box[:, 1], box[:, 2], box[:, 3]
    two = np.float32(2.0)
    xmin = np.clip(np.ceil(cx - w / two).astype(np.int32), 0, W)
    xmax = np.clip(np.ceil(cx + w / two).astype(np.int32), 0, W)
    ymin = np.clip(np.ceil(cy - h / two).astype(np.int32), 0, H)
    ymax = np.clip(np.ceil(cy + h / two).astype(np.int32), 0, H)
    out_h = (ymax - ymin).astype(np.float32)
    out_w = (xmax - xmin).astype(np.float32)
    one = np.float32(1.0)
    sy = np.where(out_h > one, np.float32(MH - 1) / np.maximum(out_h - one, one),
                  np.float32(0.0)).astype(np.float32)
    sx = np.where(out_w > one, np.float32(MW - 1) / np.maximum(out_w - one, one),
                  np.float32(0.0)).astype(np.float32)

    ys = np.arange(H, dtype=np.float32)
    xs = np.arange(W, dtype=np.float32)
    src_y = (ys[None, :] - ymin[:, None].astype(np.float32)) * sy[:, None]
    src_x = (xs[None, :] - xmin[:, None].astype(np.float32)) * sx[:, None]
    src_y = np.clip(src_y, np.float32(0.0), np.float32(MH - 1)).astype(np.float32)
    src_x = np.clip(src_x, np.float32(0.0), np.float32(MW - 1)).astype(np.float32)

    y0 = np.floor(src_y).astype(np.int32)
    y1 = np.minimum(y0 + 1, MH - 1)
    wy = (src_y - y0.astype(np.float32)).astype(np.float32)
    x0 = np.floor(src_x).astype(np.int32)
    x1 = np.minimum(x0 + 1, MW - 1)
    wx = (src_x - x0.astype(np.float32)).astype(np.float32)

    keep_y = ((ys[None, :] >= ymin[:, None].astype(np.float32))
              & (ys[None, :] < ymax[:, None].astype(np.float32))
              & valid[:, None]).astype(np.float32)
    keep_x = ((xs[None, :] >= xmin[:, None].astype(np.float32))
              & (xs[None, :] < xmax[:, None].astype(np.float32))).astype(np.float32)

    m = np.arange(MH, dtype=np.int32)
    Wy = ((m[None, None, :] == y0[:, :, None]) * (one - wy[:, :, None])
          + (m[None, None, :] == y1[:, :, None]) * wy[:, :, None]).astype(np.float32)
    Wy *= keep_y[:, :, None]
    Wx = ((m[None, :, None] == x0[:, None, :]) * (one - wx[:, None, :])
          + (m[None, :, None] == x1[:, None, :]) * wx[:, None, :]).astype(np.float32)
    Wx *= keep_x[:, None, :]
    return Wy, Wx, xmin, ymin, xmax, ymax


class Plan:
    """Sorted assignment + shapes shared by prepare, build and assemble."""

    def __init__(self, det_outs):
        Wy, Wx, xmin, ymin, xmax, ymax = _box_weight_matrices(det_outs)
        self.Wy, self.Wx = Wy, Wx
        self.xmin, self.ymin = xmin, ymin
        ph = (ymax - ymin).astype(np.int64)
        pw = (xmax - xmin).astype(np.int64)
        self.ph, self.pw = ph, pw
        # sorted rank r -> box id; core k slot c holds order[8c + k]
        self.order = np.lexsort((-pw, -ph))
        slot_ph, slot_pw = [], []
        for c in range(CPC):
            blk = self.order[8 * c : 8 * (c + 1)]
            ph_c = int(ph[blk].max())
            pw_c = int(pw[blk].max())
            slot_ph.append(max(2, ph_c + (ph_c & 1)))      # even, >= 2
            slot_pw.append(max(8, (pw_c + 3) // 4 * 4))    # mult of 4, >= 8
        self.slot_ph, self.slot_pw = slot_ph, slot_pw
        self.gmeta, self.smeta, self.out_elems = _layout(slot_ph, slot_pw)


def prepare_in_maps(plan: Plan, ins_outs: np.ndarray):
    BN = B * N
    masksT = np.swapaxes(
        np.asarray(ins_outs, np.float32).reshape(BN, MH, MW), 1, 2
    )
    Wy_pad = np.zeros((BN, H + 200, MH), np.float32)
    Wy_pad[:, :H] = plan.Wy
    Wx_pad = np.zeros((BN, MW, W + 200), np.float32)
    Wx_pad[:, :, :W] = plan.Wx

    wmat = np.zeros((N_CORES, 3 * TRI, 32, FREE), np.float16)
    for c in range(CPC):
        mode = plan.gmeta[c // GQ][0]
        ph_c, pw_c, rows_c, cols_c = plan.smeta[c]
        e = c % 3
        for k in range(N_CORES):
            i = int(plan.order[8 * c + k])
            y0, x0 = int(plan.ymin[i]), int(plan.xmin[i])
            wmat[k, c, :MW, 0:pw_c] = Wx_pad[i][:, x0 : x0 + pw_c]
            # block-diagonal mask section: box e of the triple occupies
            # lhsT columns [200+32e, 200+32e+28) in its own partition rows
            wmat[k, c, :MW, 200 + 32 * e : 200 + 32 * e + MH] = masksT[i]
            wyT = Wy_pad[i][y0 : y0 + ph_c, :].T  # [MH, ph_c]
            if mode == "single":
                wmat[k, c, :MH, 296 : 296 + ph_c] = wyT
            else:
                wmat[k, c, :MH, 296 : 296 + rows_c] = wyT[:, 0::2]
                wmat[k, c, :MH, 396 : 396 + rows_c] = wyT[:, 1::2]
    # device layout [e, p, triple, FREE]: box c = 3*triple + e
    wmat = wmat.reshape(N_CORES, TRI, 3, 32, FREE).transpose(0, 2, 3, 1, 4)
    return [{"wmat": np.ascontiguousarray(wmat[k])} for k in range(N_CORES)]


def assemble(plan: Plan, core_outs: list) -> np.ndarray:
    """Paste per-box patches into zero canvases (the unshard step)."""
    full = np.zeros((B * N, H, W), np.float32)
    for k in range(N_CORES):
        buf = np.asarray(core_outs[k], np.float32)  # [out_elems]
        for c in range(CPC):
            g, cc = c // GQ, c % GQ
            mode, rows_g, cols_g, off = plan.gmeta[g]
            ph_c, pw_c, rows_c, cols_c = plan.smeta[c]
            i = int(plan.order[8 * c + k])
            ph_i, pw_i = int(plan.ph[i]), int(plan.pw[i])
            if ph_i <= 0 or pw_i <= 0:
                continue
            grp = buf[off : off + GQ * rows_g * cols_g].reshape(
                rows_g, GQ, cols_g
            )
            slot = grp[:rows_c, cc, :cols_c]
            if mode == "single":
                patch = slot
            else:
                patch = np.empty((2 * rows_c, pw_c), np.float32)
                patch[0::2] = slot[:, :pw_c]
                patch[1::2] = slot[:, pw_c:]
            y0, x0 = int(plan.ymin[i]), int(plan.xmin[i])
            full[i, y0 : y0 + ph_i, x0 : x0 + pw_i] = patch[:ph_i, :pw_i]
    return full.reshape(B, N, H, W)


def kernel(images: np.ndarray, det_outs: np.ndarray, ins_outs: np.ndarray) -> np.ndarray:
    plan = Plan(det_outs)
    nc = build_nc(plan)
    in_maps = prepare_in_maps(plan, ins_outs)
    res = run_bass_kernel_spmd(nc, in_maps, list(range(N_CORES)))
    return assemble(plan, [res.results[k]["out"] for k in range(N_CORES)])


# revision 32
# speedup vs baseline: 1.0496x; 1.0496x over previous
"""Trainium2 Bass kernel for nn_CropAndPadMask (paste instance masks into canvases).

Math: for each (b, n) box the reference output is a bilinear resize of the
28x28 mask pasted into a zero [H, W] canvas.  Every non-zero output value
lies inside the box window [ymin, ymax) x [xmin, xmax) with extent
ph = ymax-ymin <= 200, pw = xmax-xmin <= 200.  Instead of streaming 256 MiB
of mostly-zero canvases, each core computes one f16 patch per box:

    patch[b, n] = WyP[b, n] @ mask[b, n] @ WxP[b, n]

and the host pastes patches into a zero canvas while gathering (pure data
movement).  Device HBM traffic drops ~30x vs the full canvases.

Box sizes are data dependent but the SPMD program must have static shapes,
so kernel() SPECIALIZES the program to the input: the 256 boxes are sorted
by (ph, pw) descending and dealt round-robin - sorted rank 8c+k goes to
core k, slot c.  All 8 cores share slot shapes (max over the 8 near-equal
boxes) and get equal work.  Groups of GQ=8 slots share a stage/DMA shape.

Measured hardware characteristics that shaped the schedule:
- DMA engines are descriptor-rate limited (~300 ns/descriptor/engine; only
  descriptors >= ~7 KB reach the 22.5 GB/s/engine bandwidth limit), so all
  DRAM layouts are partition-row-major: one descriptor covers a whole
  partition row of a group (3-6 KB).
- PE matmul issue is LDWEIGHTS-bound (~57 + 1.18*M ns per matmul, M = lhsT
  free size), so the schedule minimizes matmul count and total output rows:
  * S = mask @ WxP for 3 boxes at a time via ONE [96, pw] matmul with a
    block-diagonal maskT lhsT (operand base partitions are limited to
    {0, 32, 64}, which caps the stacking at 3); one PSUM->SBUF copy per
    triple.
  * ph_g <= 128: the patch is ONE matmul [ph, pw] (partition = canvas row).
  * ph_g  > 128: two matmuls (even/odd canvas rows) into [ph/2, 2*pw],
    which also doubles the out-DMA descriptor width.
- PSUM->SBUF f16 casts are greedily balanced over Act/DVE (GPSIMD cannot
  access PSUM).

f16 off-PSUM everywhere: DMA bytes halved vs f32 and f16 rounding
(~4e-4 rel) is far inside the 2e-2 gate.  Group padding regions are
transferred as garbage and never read by the host.
"""

import sys

for _p in ("/opt/trn_rl_repo", "/root/.axon_site/_ro/trn_rl_repo"):
    if _p not in sys.path:
        sys.path.append(_p)

import numpy as np

import concourse.bass as bass
import concourse.mybir as mybir
import concourse.tile as tile
from concourse.bass_utils import run_bass_kernel_spmd

B, N, H, W, MH, MW = 4, 64, 512, 512, 28, 28
N_CORES = 8
CPC = (B * N) // N_CORES  # boxes per core = 32
GQ = 8                    # slots per group (one out-DMA per group; wide
                          # groups => multi-KB DMA descriptors)
NG = CPC // GQ            # groups per core = 4
FREE = 496                # fixed input row: WxP @0 | bd-maskT @200 | wyT @296
TRI = (CPC + 2) // 3      # S-triples per core (last may be ragged)


def _split_multi_waits(nc: bass.Bass) -> None:
    """The walrus bundled in this container accepts at most ONE sync wait per
    instruction.  Tile freely attaches several.  Hoist the extras onto
    standalone EventSemaphore carriers inserted just before the instruction on
    the same engine (per-engine program order makes this equivalent)."""
    n_new = 0
    for f in nc.m.functions:
        for bb in f.blocks:
            lst = bb.instructions
            i = 0
            while i < len(lst):
                ins = lst[i]
                si = ins.sync_info
                if si is not None and si.on_wait and len(si.on_wait) > 1:
                    waits = list(si.on_wait)
                    ins.sync_info = mybir.SyncInfo(
                        on_wait=waits[:1], on_update=list(si.on_update or [])
                    )
                    carriers = []
                    for w in waits[1:]:
                        n_new += 1
                        carriers.append(
                            mybir.InstEventSemaphore(
                                name=f"I-waitsplit-{n_new}",
                                ins=[],
                                outs=[],
                                engine=ins.engine,
                                sync_info=mybir.SyncInfo(on_wait=[w], on_update=[]),
                            )
                        )
                    lst[i:i] = carriers
                    i += len(carriers)
                i += 1


def _layout(slot_ph, slot_pw):
    """Per-slot quantized shapes -> group meta + per-slot meta.

    Group g (GQ slots) shares (mode, rows_g, cols_g) for its stage tile and
    out-DMA; each slot keeps its own matmul/copy extents (rows_c, cols_c).
    """
    groups = []
    slots = []
    off = 0
    for g in range(CPC // GQ):
        sl = slice(GQ * g, GQ * (g + 1))
        ph_g = max(slot_ph[sl])
        eo = ph_g > 128
        rows_g = cols_g = 0
        for c in range(GQ * g, GQ * (g + 1)):
            ph_c, pw_c = slot_ph[c], slot_pw[c]
            if eo:
                rows_c, cols_c = ph_c // 2, 2 * pw_c
            else:
                rows_c, cols_c = ph_c, pw_c
            slots.append((ph_c, pw_c, rows_c, cols_c))
            rows_g = max(rows_g, rows_c)
            cols_g = max(cols_g, cols_c)
        groups.append(("eo" if eo else "single", rows_g, cols_g, off))
        off += GQ * rows_g * cols_g
    return groups, slots, off


def build_nc(plan, cpc: int = CPC) -> bass.Bass:
    """One core's program, specialized to the plan's slot/group shapes."""
    f16 = mybir.dt.float16
    f32 = mybir.dt.float32
    groups, slots, out_elems = plan.gmeta, plan.smeta, plan.out_elems
    ng = len(groups)
    nc = bass.Bass()
    wmat = nc.dram_tensor(
        "wmat", [3, 32, TRI, FREE], f16, kind="ExternalInput"
    )
    out = nc.dram_tensor("out", [out_elems], f16, kind="ExternalOutput")

    with tile.TileContext(nc) as tc:
        with (
            tc.tile_pool(name="win", bufs=3) as win_pool,
            tc.tile_pool(name="ssb", bufs=5) as s_pool,
            tc.tile_pool(name="stage", bufs=3) as stage_pool,
            tc.tile_pool(name="psum_s", bufs=4, space="PSUM") as psum_s,
            tc.tile_pool(name="psum_c", bufs=4, space="PSUM") as psum_c,
        ):
            # input loads: box c = (triple t = c//3, e = c%3) lives at SBUF
            # partition 32*e + p of triple-tile column t.  wmat is stored
            # partition-major ([3, 32, TRI, FREE]) so each load descriptor
            # covers a whole (triples x FREE) run per partition.
            load_plan = [
                (0, 1, "split"),
                (1, 3, "sync"),
                (4, TRI - 4, "gpsimd"),
            ]
            win_of: dict[int, tuple[object, int]] = {}
            for t0, tlen, eng in load_plan:
                w_t = win_pool.tile([96, tlen, FREE], f16)
                src = wmat[:, :, t0 : t0 + tlen, :].rearrange(
                    "e p t f -> (e p) t f"
                )
                if eng == "split":
                    # the first triple gates the whole pipeline: halve its
                    # latency by splitting partitions over both HWDGE queues
                    nc.scalar.dma_start(w_t[0:48], src[0:48])
                    nc.sync.dma_start(w_t[48:96], src[48:96])
                else:
                    getattr(nc, eng).dma_start(w_t[:], src)
                for tt in range(tlen):
                    win_of[t0 + tt] = (w_t, tt)

            s_of: dict[int, object] = {}
            eng_load = [0.0, 0.0]  # projected busy ns: [Act, DVE]

            def copy_rr(dst, src, free):
                """Greedy-balance PSUM->SBUF copies over the two PSUM-capable
                engines (GPSIMD cannot access PSUM)."""
                est = ((free + 172) * 0.833, (free + 120) * 1.042)
                if eng_load[0] + est[0] <= eng_load[1] + est[1]:
                    nc.scalar.copy(dst, src)
                    eng_load[0] += est[0]
                else:
                    nc.vector.tensor_copy(dst, src)
                    eng_load[1] += est[1]

            def s_triple(t):
                """Block-diag S = mask @ WxP for the <=3 boxes of triple t:
                ONE matmul [96, pw_t] (lhsT = block-diagonal maskT) and ONE
                PSUM->SBUF copy serve all three boxes."""
                if t >= TRI or t in s_of:
                    return
                cs = [c for c in (3 * t, 3 * t + 1, 3 * t + 2) if c < cpc]
                pw_t = max(slots[c][1] for c in cs)
                w_t, tt = win_of[t]
                s_p = psum_s.tile([96, pw_t], f32)
                nc.tensor.matmul(
                    s_p[:, :],
                    w_t[0:96, tt, 200:296],
                    w_t[0:96, tt, 0:pw_t],
                    start=True,
                    stop=True,
                )
                s_sb = s_pool.tile([96, pw_t], f16)
                copy_rr(s_sb[:], s_p[:], pw_t)
                s_of[t] = s_sb

            PIPE = 2  # triples of S lookahead
            for t in range(PIPE):
                s_triple(t)
            for k in range(ng):
                mode, rows_g, cols_g, off = groups[k]
                stage = stage_pool.tile([rows_g, GQ, cols_g], f16, tag="stage")
                for q in range(GQ):
                    c = GQ * k + q
                    j, e = c // 3, c % 3
                    s_triple(j + PIPE)
                    w_t, jj = win_of[j]
                    s_sb = s_of[j]
                    b0 = 32 * e
                    ph_c, pw_c, rows_c, cols_c = slots[c]
                    if mode == "single":
                        p_c = psum_c.tile([rows_c, cols_c], f32)
                        nc.tensor.matmul(
                            p_c[:, :],
                            w_t[b0 : b0 + MW, jj, 296 : 296 + ph_c],
                            s_sb[b0 : b0 + MW, 0:pw_c],
                            start=True,
                            stop=True,
                        )
                        copy_rr(stage[0:rows_c, q, 0:cols_c], p_c[:], cols_c)
                    else:
                        p_c = psum_c.tile([rows_c, cols_c], f32)
                        nc.tensor.matmul(
                            p_c[:, 0:pw_c],
                            w_t[b0 : b0 + MW, jj, 296 : 296 + rows_c],
                            s_sb[b0 : b0 + MW, 0:pw_c],
                            start=True,
                            stop=True,
                        )
                        nc.tensor.matmul(
                            p_c[:, pw_c : 2 * pw_c],
                            w_t[b0 : b0 + MW, jj, 396 : 396 + rows_c],
                            s_sb[b0 : b0 + MW, 0:pw_c],
                            start=True,
                            stop=True,
                        )
                        copy_rr(stage[0:rows_c, q, 0:cols_c], p_c[:], cols_c)
                # partition-row-major DRAM layout: one descriptor per
                # partition covering all GQ boxes' row p (GQ*cols_g*2 bytes)
                out_ap = out[off : off + GQ * rows_g * cols_g].rearrange(
                    "(p c j) -> p c j", c=GQ, p=rows_g
                )
                nc.sync.dma_start(out_ap, stage[:])
    _split_multi_waits(nc)
    return nc


def _box_weight_matrices(det_outs: np.ndarray):
    """Wy [BN, H, MH], Wx [BN, MW, W] (f32) + box corners, reference semantics."""
    det = np.asarray(det_outs, dtype=np.float32).reshape(B * N, 6)
    score = det[:, 5]
    thr = np.float32(50.0) if np.max(score) > 50.0 else np.float32(-100.0)
    valid = score >= thr
    box = np.maximum(det, np.float32(1.0))
    cx, cy, w, h = box[:, 0], box[:, 1], box[:, 2], box[:, 3]
    two = np.float32(2.0)
    xmin = np.clip(np.ceil(cx - w / two).astype(np.int32), 0, W)
    xmax = np.clip(np.ceil(cx + w / two).astype(np.int32), 0, W)
    ymin = np.clip(np.ceil(cy - h / two).astype(np.int32), 0, H)
    ymax = np.clip(np.ceil(cy + h / two).astype(np.int32), 0, H)
    out_h = (ymax - ymin).astype(np.float32)
    out_w = (xmax - xmin).astype(np.float32)
    one = np.float32(1.0)
    sy = np.where(out_h > one, np.float32(MH - 1) / np.maximum(out_h - one, one),
                  np.float32(0.0)).astype(np.float32)
    sx = np.where(out_w > one, np.float32(MW - 1) / np.maximum(out_w - one, one),
                  np.float32(0.0)).astype(np.float32)

    ys = np.arange(H, dtype=np.float32)
    xs = np.arange(W, dtype=np.float32)
    src_y = (ys[None, :] - ymin[:, None].astype(np.float32)) * sy[:, None]
    src_x = (xs[None, :] - xmin[:, None].astype(np.float32)) * sx[:, None]
    src_y = np.clip(src_y, np.float32(0.0), np.float32(MH - 1)).astype(np.float32)
    src_x = np.clip(src_x, np.float32(0.0), np.float32(MW - 1)).astype(np.float32)

    y0 = np.floor(src_y).astype(np.int32)
    y1 = np.minimum(y0 + 1, MH - 1)
    wy = (src_y - y0.astype(np.float32)).astype(np.float32)
    x0 = np.floor(src_x).astype(np.int32)
    x1 = np.minimum(x0 + 1, MW - 1)
    wx = (src_x - x0.astype(np.float32)).astype(np.float32)

    keep_y = ((ys[None, :] >= ymin[:, None].astype(np.float32))
              & (ys[None, :] < ymax[:, None].astype(np.float32))
              & valid[:, None]).astype(np.float32)
    keep_x = ((xs[None, :] >= xmin[:, None].astype(np.float32))
              & (xs[None, :] < xmax[:, None].astype(np.float32))).astype(np.float32)

    m = np.arange(MH, dtype=np.int32)
    Wy = ((m[None, None, :] == y0[:, :, None]) * (one - wy[:, :, None])
          + (m[None, None, :] == y1[:, :, None]) * wy[:, :, None]).astype(np.float32)
    Wy *= keep_y[:, :, None]
    Wx = ((m[None, :, None] == x0[:, None, :]) * (one - wx[:, None, :])
          + (m[None, :, None] == x1[:, None, :]) * wx[:, None, :]).astype(np.float32)
    Wx *= keep_x[:, None, :]
    return Wy, Wx, xmin, ymin, xmax, ymax


class Plan:
    """Sorted assignment + shapes shared by prepare, build and assemble."""

    def __init__(self, det_outs):
        Wy, Wx, xmin, ymin, xmax, ymax = _box_weight_matrices(det_outs)
        self.Wy, self.Wx = Wy, Wx
        self.xmin, self.ymin = xmin, ymin
        ph = (ymax - ymin).astype(np.int64)
        pw = (xmax - xmin).astype(np.int64)
        self.ph, self.pw = ph, pw
        # sorted rank r -> box id; core k slot c holds order[8c + k]
        self.order = np.lexsort((-pw, -ph))
        slot_ph, slot_pw = [], []
        for c in range(CPC):
            blk = self.order[8 * c : 8 * (c + 1)]
            ph_c = int(ph[blk].max())
            pw_c = int(pw[blk].max())
            slot_ph.append(max(2, ph_c + (ph_c & 1)))      # even, >= 2
            slot_pw.append(max(8, (pw_c + 3) // 4 * 4))    # mult of 4, >= 8
        self.slot_ph, self.slot_pw = slot_ph, slot_pw
        self.gmeta, self.smeta, self.out_elems = _layout(slot_ph, slot_pw)


def prepare_in_maps(plan: Plan, ins_outs: np.ndarray):
    BN = B * N
    masksT = np.swapaxes(
        np.asarray(ins_outs, np.float32).reshape(BN, MH, MW), 1, 2
    )
    Wy_pad = np.zeros((BN, H + 200, MH), np.float32)
    Wy_pad[:, :H] = plan.Wy
    Wx_pad = np.zeros((BN, MW, W + 200), np.float32)
    Wx_pad[:, :, :W] = plan.Wx

    wmat = np.zeros((N_CORES, 3 * TRI, 32, FREE), np.float16)
    for c in range(CPC):
        mode = plan.gmeta[c // GQ][0]
        ph_c, pw_c, rows_c, cols_c = plan.smeta[c]
        e = c % 3
        for k in range(N_CORES):
            i = int(plan.order[8 * c + k])
            y0, x0 = int(plan.ymin[i]), int(plan.xmin[i])
            wmat[k, c, :MW, 0:pw_c] = Wx_pad[i][:, x0 : x0 + pw_c]
            # block-diagonal mask section: box e of the triple occupies
            # lhsT columns [200+32e, 200+32e+28) in its own partition rows
            wmat[k, c, :MW, 200 + 32 * e : 200 + 32 * e + MH] = masksT[i]
            wyT = Wy_pad[i][y0 : y0 + ph_c, :].T  # [MH, ph_c]
            if mode == "single":
                wmat[k, c, :MH, 296 : 296 + ph_c] = wyT
            else:
                wmat[k, c, :MH, 296 : 296 + rows_c] = wyT[:, 0::2]
                wmat[k, c, :MH, 396 : 396 + rows_c] = wyT[:, 1::2]
    # device layout [e, p, triple, FREE]: box c = 3*triple + e
    wmat = wmat.reshape(N_CORES, TRI, 3, 32, FREE).transpose(0, 2, 3, 1, 4)
    return [{"wmat": np.ascontiguousarray(wmat[k])} for k in range(N_CORES)]


def assemble(plan: Plan, core_outs: list) -> np.ndarray:
    """Paste per-box patches into zero canvases (the unshard step)."""
    full = np.zeros((B * N, H, W), np.float32)
    for k in range(N_CORES):
        buf = np.asarray(core_outs[k], np.float32)  # [out_elems]
        for c in range(CPC):
            g, cc = c // GQ, c % GQ
            mode, rows_g, cols_g, off = plan.gmeta[g]
            ph_c, pw_c, rows_c, cols_c = plan.smeta[c]
            i = int(plan.order[8 * c + k])
            ph_i, pw_i = int(plan.ph[i]), int(plan.pw[i])
            if ph_i <= 0 or pw_i <= 0:
                continue
            grp = buf[off : off + GQ * rows_g * cols_g].reshape(
                rows_g, GQ, cols_g
            )
            slot = grp[:rows_c, cc, :cols_c]
            if mode == "single":
                patch = slot
            else:
                patch = np.empty((2 * rows_c, pw_c), np.float32)
                patch[0::2] = slot[:, :pw_c]
                patch[1::2] = slot[:, pw_c:]
            y0, x0 = int(plan.ymin[i]), int(plan.xmin[i])
            full[i, y0 : y0 + ph_i, x0 : x0 + pw_i] = patch[:ph_i, :pw_i]
    return full.reshape(B, N, H, W)


def kernel(images: np.ndarray, det_outs: np.ndarray, ins_outs: np.ndarray) -> np.ndarray:
    plan = Plan(det_outs)
    nc = build_nc(plan)
    in_maps = prepare_in_maps(plan, ins_outs)
    res = run_bass_kernel_spmd(nc, in_maps, list(range(N_CORES)))
    return assemble(plan, [res.results[k]["out"] for k in range(N_CORES)])
